# revision 6
# baseline (speedup 1.0000x reference)
"""Causal self-attention TRN2 Bass kernel (B=4, T=2048, C=1024, H=16, D=64, fp32).

Sharding: 8 cores = 4 batches x 2 head-groups (8 heads each). Each core computes
its batch's QKV for its heads, causal flash-style attention, and a partial
output projection; the host sums the two head-group partials per batch.

Device dataflow (all matmuls in float32r — ~1.5e-4 rel, 4x fp32 speed):
  phase 1: qkT = [Wq|Wk]^T x  (via lhsT=W blocks, rhs=x^T chunks) -> DRAM scratch
           v   = x Wv         (via lhsT=x^T blocks, rhs=Wv)       -> DRAM scratch
  phase 2: per head-pair: S^T[k,q] = K^T.T Q^T (row-tiled 2 heads in PE),
           causal mask only on diagonal 128x128 blocks, exp on ACT (scale=1/8),
           O^T/sums fused: lhsT=[V|ones] so psum rows 0..63=O^T, 64..127=sums,
           divide via DVE reciprocal+mult -> o^T in SBUF.
  phase 3: y^T = W_proj^T o^T (partial over this core's heads) -> DRAM out.
Host: y[b] = (yT[2b] + yT[2b+1]).T
"""

import numpy as np
from contextlib import ExitStack

import concourse.bass as bass
import concourse.tile as tile
from concourse import bacc, mybir
from concourse.bass import ts
from concourse.bass_utils import run_bass_kernel_spmd

N_CORES = 8
B, T, C, H, D = 4, 2048, 1024, 16, 64
CB = C // 128          # 8 contraction blocks
NKB = T // 128         # 16 key blocks
NQC = T // 512         # 4 query chunks
NEG = -1.0e9

F32 = mybir.dt.float32
F32R = mybir.dt.float32r
AF = mybir.ActivationFunctionType
OP = mybir.AluOpType

_CACHE = {}


def _build():
    nc = bacc.Bacc("TRN2", target_bir_lowering=False, debug=False, num_devices=N_CORES)

    xT = nc.dram_tensor("xT", [C, T], F32R, kind="ExternalInput").ap()
    w_qk = nc.dram_tensor("w_qk", [C, 1024], F32R, kind="ExternalInput").ap()
    w_v = nc.dram_tensor("w_v", [C, 512], F32R, kind="ExternalInput").ap()
    w_pr = nc.dram_tensor("w_pr", [512, C], F32R, kind="ExternalInput").ap()
    b_qk = nc.dram_tensor("b_qk", [1024], F32, kind="ExternalInput").ap()
    b_v = nc.dram_tensor("b_v", [128, 512], F32, kind="ExternalInput").ap()
    b_pr = nc.dram_tensor("b_pr", [C], F32, kind="ExternalInput").ap()
    yT = nc.dram_tensor("yT", [C, T], F32, kind="ExternalOutput").ap()

    xT_r = xT.rearrange("(cb p) t -> p cb t", p=128)
    w_qk_r = w_qk.rearrange("(cb p) m -> p cb m", p=128)
    w_v_r = w_v.rearrange("(cb p) m -> p cb m", p=128)
    w_pr_r = w_pr.rearrange("(pb p) m -> p pb m", p=128)
    b_qk_r = b_qk.rearrange("(m p) -> p m", p=128)
    b_pr_r = b_pr.rearrange("(m p) -> p m", p=128)

    with tile.TileContext(nc) as tc:
        with ExitStack() as ctx:
            # pools
            io = ctx.enter_context(tc.tile_pool(name="io", bufs=2))        # 16KB tiles
            wqk_p = ctx.enter_context(tc.tile_pool(name="wqk", bufs=1))
            w2_p = ctx.enter_context(tc.tile_pool(name="w2", bufs=1))      # w_v <-> w_pr
            qkt_p = ctx.enter_context(tc.tile_pool(name="qkt", bufs=4))
            stage_p = ctx.enter_context(tc.tile_pool(name="stage", bufs=4))
            p_p = ctx.enter_context(tc.tile_pool(name="pp", bufs=6))
            ot_p = ctx.enter_context(tc.tile_pool(name="ot", bufs=1))
            misc = ctx.enter_context(tc.tile_pool(name="misc", bufs=1))
            rec_p = ctx.enter_context(tc.tile_pool(name="rec", bufs=4))
            dram = ctx.enter_context(tc.tile_pool(name="dram", bufs=1, space="DRAM"))
            ps_qkv = ctx.enter_context(tc.tile_pool(name="ps_qkv", bufs=2, space="PSUM"))
            ps_s_p = ctx.enter_context(tc.tile_pool(name="ps_s", bufs=4, space="PSUM"))
            ps_o_p = ctx.enter_context(tc.tile_pool(name="ps_o", bufs=2, space="PSUM"))

            # constants
            b_qk_sb = misc.tile([128, 8], F32)
            nc.sync.dma_start(b_qk_sb[:], b_qk_r)
            b_v_sb = misc.tile([128, 512], F32)
            nc.sync.dma_start(b_v_sb[:], b_v)
            b_pr_sb = misc.tile([128, 8], F32)
            nc.sync.dma_start(b_pr_sb[:], b_pr_r)
            ones_sb = misc.tile([128, 64], F32)
            nc.gpsimd.memset(ones_sb[:], 1.0)
            tri = misc.tile([128, 128], F32)
            nc.gpsimd.memset(tri[:], 0.0)
            # 0 where q(free) >= k(partition), NEG where q < k
            nc.gpsimd.affine_select(
                out=tri[:], in_=tri[:], compare_op=OP.is_ge, fill=NEG,
                base=0, pattern=[[1, 128]], channel_multiplier=-1,
            )

            # DRAM scratch
            qkT_d = dram.tile([8, 128, T], F32R)
            v_d = dram.tile([NKB, 128, 512], F32R)

            # weights
            w_qk_sb = wqk_p.tile([128, CB, 1024], F32R)
            nc.sync.dma_start(w_qk_sb[:], w_qk_r)
            w_v_sb = w2_p.tile([128, CB, 512], F32R, tag="w16k")
            nc.sync.dma_start(w_v_sb[:], w_v_r)

            # ---------- phase 1: QKV ----------
            for tch in range(4):
                x_t = io.tile([128, CB, 512], F32R, tag="io16k", name=f"x_{tch}")
                nc.sync.dma_start(x_t[:], xT_r[:, :, ts(tch, 512)])
                for m in range(8):
                    ps = ps_qkv.tile([128, 512], F32, tag="ps_qkv", name=f"qk_{tch}_{m}")
                    for cb in range(CB):
                        nc.tensor.matmul(
                            ps[:], w_qk_sb[:, cb, ts(m, 128)], x_t[:, cb],
                            start=(cb == 0), stop=(cb == CB - 1),
                        )
                    st = stage_p.tile([128, 512], F32R, tag="stage", name=f"qks_{tch}_{m}")
                    nc.scalar.activation(st[:], ps[:], AF.Identity, bias=b_qk_sb[:, m : m + 1])
                    nc.sync.dma_start(qkT_d[m, :, ts(tch, 512)], st[:])
                for tb4 in range(4):
                    tb = tch * 4 + tb4
                    ps = ps_qkv.tile([128, 512], F32, tag="ps_qkv", name=f"v_{tb}")
                    for cb in range(CB):
                        nc.tensor.matmul(
                            ps[:], x_t[:, cb, ts(tb4, 128)], w_v_sb[:, cb],
                            start=(cb == 0), stop=(cb == CB - 1),
                        )
                    st = stage_p.tile([128, 512], F32R, tag="stage", name=f"vs_{tb}")
                    nc.vector.tensor_tensor(st[:], ps[:], b_v_sb[:], OP.add)
                    nc.sync.dma_start(v_d[tb], st[:])

            # ---------- phase 2: attention, one head-pair at a time ----------
            oT = ot_p.tile([128, 4, T], F32R, name="oT")
            for pr in range(4):
                qT = qkt_p.tile([128, T], F32R, tag="qkt", name=f"qT_{pr}")
                nc.sync.dma_start(qT[:], qkT_d[pr])
                kT = qkt_p.tile([128, T], F32R, tag="qkt", name=f"kT_{pr}")
                nc.sync.dma_start(kT[:], qkT_d[4 + pr])
                v_aug = io.tile([128, NKB, 2, 128], F32R, tag="io16k", name=f"va_{pr}")
                for j in (0, 1):
                    nc.sync.dma_start(
                        v_aug[:, :, j, 0:64],
                        v_d[:, :, pr * 128 + j * 64 : pr * 128 + (j + 1) * 64]
                        .rearrange("kb p d -> p kb d"),
                    )
                nc.vector.tensor_copy(
                    v_aug[:, :, :, 64:128],
                    ones_sb[:, None, None, :].to_broadcast((128, NKB, 2, 64)),
                )

                for qc in range(NQC):
                    nkb = 4 * qc + 4
                    ps_o = [
                        ps_o_p.tile([128, 512], F32, tag="ps_o", name=f"o_{pr}_{qc}_{j}")
                        for j in (0, 1)
                    ]

                    # software-pipelined: issue S(kb+1) before AV(kb)
                    ps_s = [[None, None] for _ in range(nkb)]

                    def s_step(kb):
                        r = kb - 4 * qc
                        qlo = 128 * r if r > 0 else 0
                        for j in (0, 1):
                            pb = j * 64
                            s = ps_s_p.tile([128, 512], F32, tag="ps_s",
                                            name=f"s_{pr}_{qc}_{kb}_{j}")
                            nc.tensor.matmul(
                                s[:, qlo:512],
                                kT[pb : pb + 64, ts(kb, 128)],
                                qT[pb : pb + 64, qc * 512 + qlo : (qc + 1) * 512],
                                start=True, stop=True, tile_position=(pb, 0),
                            )
                            if r >= 0:
                                nc.vector.tensor_tensor(
                                    s[:, qlo : qlo + 128], s[:, qlo : qlo + 128],
                                    tri[:], OP.add,
                                )
                            ps_s[kb][j] = s

                    def av_step(kb):
                        r = kb - 4 * qc
                        qlo = 128 * r if r > 0 else 0
                        for j in (0, 1):
                            p_t = p_p.tile([128, 512], F32R, tag="p",
                                           name=f"p_{pr}_{qc}_{kb}_{j}")
                            nc.scalar.activation(
                                p_t[:, qlo:512], ps_s[kb][j][:, qlo:512],
                                AF.Exp, scale=0.125,
                            )
                            nc.tensor.matmul(
                                ps_o[j][:, qlo:512], v_aug[:, kb, j], p_t[:, qlo:512],
                                start=(kb == 0), stop=(kb == nkb - 1),
                            )

                    s_step(0)
                    for kb in range(nkb):
                        if kb + 1 < nkb:
                            s_step(kb + 1)
                        av_step(kb)

                    for j in (0, 1):
                        rec = rec_p.tile([64, 512], F32, tag="rec",
                                         name=f"rec_{pr}_{qc}_{j}")
                        nc.vector.reciprocal(rec[:], ps_o[j][64:128, :])
                        nc.vector.tensor_tensor(
                            oT[j * 64 : (j + 1) * 64, pr, ts(qc, 512)],
                            ps_o[j][0:64, :], rec[:], OP.mult,
                        )

            # ---------- phase 3: projection ----------
            w_pr_sb = w2_p.tile([128, 4, 1024], F32R, tag="w16k", name="w_pr_sb")
            nc.sync.dma_start(w_pr_sb[:], w_pr_r)
            for m in range(8):
                for tch in range(4):
                    ps = ps_qkv.tile([128, 512], F32, tag="ps_qkv", name=f"y_{m}_{tch}")
                    for pb in range(4):
                        nc.tensor.matmul(
                            ps[:], w_pr_sb[:, pb, ts(m, 128)], oT[:, pb, ts(tch, 512)],
                            start=(pb == 0), stop=(pb == 3),
                        )
                    st = stage_p.tile([128, 512], F32, tag="stage", name=f"ys_{m}_{tch}")
                    nc.scalar.activation(st[:], ps[:], AF.Identity, bias=b_pr_sb[:, m : m + 1])
                    nc.sync.dma_start(yT.rearrange("(m p) t -> p m t", p=128)[:, m, ts(tch, 512)], st[:])

    nc.compile()
    return nc


def _in_maps(x, W_attn, b_attn, W_proj, b_proj):
    maps = []
    for b in range(B):
        for g in range(2):
            cs = slice(g * 512, (g + 1) * 512)
            maps.append({
                "xT": np.ascontiguousarray(x[b].T),
                "w_qk": np.ascontiguousarray(
                    np.concatenate([W_attn[:, cs], W_attn[:, 1024 + cs.start : 1024 + cs.stop]], axis=1)),
                "w_v": np.ascontiguousarray(W_attn[:, 2048 + cs.start : 2048 + cs.stop]),
                "w_pr": np.ascontiguousarray(W_proj[cs, :]),
                "b_qk": np.ascontiguousarray(
                    np.concatenate([b_attn[cs], b_attn[1024 + cs.start : 1024 + cs.stop]])),
                "b_v": np.ascontiguousarray(
                    np.tile(b_attn[2048 + cs.start : 2048 + cs.stop][None, :], (128, 1))),
                "b_pr": np.ascontiguousarray(b_proj),
            })
    return maps


def kernel(x, W_attn, b_attn, W_proj, b_proj):
    x = np.asarray(x, dtype=np.float32)
    W_attn = np.asarray(W_attn, dtype=np.float32)
    b_attn = np.asarray(b_attn, dtype=np.float32)
    W_proj = np.asarray(W_proj, dtype=np.float32)
    b_proj = np.asarray(b_proj, dtype=np.float32)

    if "nc" not in _CACHE:
        _CACHE["nc"] = _build()
    nc = _CACHE["nc"]

    maps = _in_maps(x, W_attn, b_attn, W_proj, b_proj)
    res = run_bass_kernel_spmd(nc, maps, core_ids=list(range(N_CORES)))
    y = np.empty((B, T, C), dtype=np.float32)
    for b in range(B):
        y[b] = (res.results[2 * b]["yT"] + res.results[2 * b + 1]["yT"]).T
    return y
